# revision 1
# baseline (speedup 1.0000x reference)
"""Trainium2 Bass kernel for 2-layer LSTM (H=16) time-series predictor.

Model (reference): x:[B,T] -> per-t scalar input into LSTMCell1(1->16) ->
LSTMCell2(16->16), teacher-forced over T steps, then head(h2)=fc2(leaky(fc1(h2)))
produces out[:,0]; 32 autoregressive steps feed head output back as input.
Output [B, 33].

Sharding: data-parallel over batch across 8 cores (256 batch each), weights
replicated. Per-core layout: hidden dim on partitions, batch on the free dim.

Hardware constraints that shape the design:
  * every vector/scalar instruction needs ALL operands on the same partition
    range -> every elementwise tensor (c, h, sigmoid/tanh outputs) lives on
    partition window 0:32 ([layer1; layer2] stacked), and the 4 gate types are
    separated along the PSUM *free* dim instead of partitions;
  * a matmul writes one [M<=32-aligned, N<=512] PSUM block -> 4 matmuls per
    step, one per gate type (i, f, o, g), each [32, 256];
  * layer2 lags one step so both layers' gates use the same h1 and one shared
    rhs: a rotating hx buffer [h1(16); h2(16); x_t(1); ones(1)], whose x/ones
    rows are DMA-prefilled straight from DRAM several steps ahead.
"""

import numpy as np

import concourse.bass as bass
import concourse.tile as tile
from concourse import bacc, mybir
from concourse.bass_utils import run_bass_kernel_spmd

F32 = mybir.dt.float32
AF = mybir.ActivationFunctionType

H = 16
B = 2048
T = 2048
FUT = 32
NCORES = 8
BC = B // NCORES  # 256 batch per core
NHX = 2           # rotation depth of the hx rhs buffers

# torch gate row order in the 4H weight matrices: i, f, g, o
_G = {"i": slice(0, H), "f": slice(H, 2 * H), "g": slice(2 * H, 3 * H), "o": slice(3 * H, 4 * H)}
# our gate order along the psum free dim / lhsT column blocks
_ORDER = ["i", "f", "o", "g"]


def _pack_weights(W_ih1, W_hh1, b_ih1, b_hh1, W_ih2, W_hh2, b_ih2, b_hh2,
                  fc1_w, fc1_b, fc2_w, fc2_b):
    b1 = b_ih1 + b_hh1  # [64]
    b2 = b_ih2 + b_hh2

    # main loop lhsTs; column block k (32 wide) = gate _ORDER[k], [l1(16)|l2(16)].
    # main_h rows = [h1(16); h2(16)]; main_x rows = [x(1); ones(1)].
    main_h = np.zeros((32, 128), np.float32)
    main_x = np.zeros((2, 128), np.float32)
    for k, gn in enumerate(_ORDER):
        c0 = 32 * k
        main_h[0:16, c0:c0 + 16] = W_hh1[_G[gn], :].T      # h1 -> layer1 gate
        main_h[0:16, c0 + 16:c0 + 32] = W_ih2[_G[gn], :].T  # h1 -> layer2 gate
        main_h[16:32, c0 + 16:c0 + 32] = W_hh2[_G[gn], :].T  # h2 -> layer2 gate
        main_x[0, c0:c0 + 16] = W_ih1[_G[gn], 0]            # x  -> layer1 gate
        main_x[1, c0:c0 + 16] = b1[_G[gn]]
        main_x[1, c0 + 16:c0 + 32] = b2[_G[gn]]

    # rollout layer1: split into K=1 part (input o) and K=17 part (h1 + bias)
    ro1x = np.zeros((1, 64), np.float32)   # columns: 4 gate blocks of 16
    ro1h = np.zeros((17, 64), np.float32)
    ro2h1 = np.zeros((17, 64), np.float32)  # h1 + bias part of layer2
    ro2h2 = np.zeros((16, 64), np.float32)  # h2 part of layer2
    for k, gn in enumerate(_ORDER):
        c0 = 16 * k
        ro1x[0, c0:c0 + 16] = W_ih1[_G[gn], 0]
        ro1h[0:16, c0:c0 + 16] = W_hh1[_G[gn], :].T
        ro1h[16, c0:c0 + 16] = b1[_G[gn]]
        ro2h1[0:16, c0:c0 + 16] = W_ih2[_G[gn], :].T
        ro2h1[16, c0:c0 + 16] = b2[_G[gn]]
        ro2h2[0:16, c0:c0 + 16] = W_hh2[_G[gn], :].T

    ro_fc1 = np.zeros((17, 8), np.float32)
    ro_fc1[0:16] = fc1_w.T  # fc1_w [8,16]
    ro_fc1[16] = fc1_b

    # M=1 matmuls misbehave on HW — replicate the fc2 column into M=8 and
    # read row 0 of the result instead
    ro_fc2 = np.zeros((9, 8), np.float32)
    ro_fc2[0:8] = fc2_w.T  # fc2_w [1,8]
    ro_fc2[8] = fc2_b

    return dict(main_h=main_h, main_x=main_x, ro1x=ro1x, ro1h=ro1h,
                ro2h1=ro2h1, ro2h2=ro2h2, ro_fc1=ro_fc1, ro_fc2=ro_fc2)


def _pack_x(x_core, t_steps):
    """x_core [BC, t] -> [t+1, 2, BC]: per step a [x_t; 1] pair (last x row 0)."""
    xe = np.ones((t_steps + 1, 2, x_core.shape[0]), np.float32)
    xe[:, 0, :] = 0.0
    xe[:t_steps, 0, :] = x_core.T.astype(np.float32)
    return xe


# ---------------------------------------------------------------------------
# device kernel
# ---------------------------------------------------------------------------

def _build(t_steps=T, fut=FUT, bc=BC, dbg=False, loop_steps=None):
    # loop_steps < t_steps runs fewer recurrence steps with identical I/O
    # sizes — for isolating device time via wall-clock deltas (output is
    # mathematically meaningless in that mode)
    nc = bacc.Bacc("TRN2", target_bir_lowering=False)
    dbg_d = {}
    if dbg == 3:
        for name, p in [("d_zt", 9), ("d_z", 8), ("d_ops", 8)]:
            dbg_d[name] = nc.dram_tensor(name, [p, bc], F32, kind="ExternalOutput")
        dbg_d["d_osb"] = nc.dram_tensor("d_osb", [1, fut + 1, bc], F32, kind="ExternalOutput")
    elif dbg:
        for name, p in [("d_h1e", 17), ("d_h2e", 17), ("d_rc1", 16), ("d_rc2", 16)]:
            dbg_d[name] = nc.dram_tensor(name, [p, bc], F32, kind="ExternalOutput")

    xe_d = nc.dram_tensor("xe", [t_steps + 1, 2, bc], F32, kind="ExternalInput")
    w_d = {}
    for name, shape in [("main_h", [32, 128]), ("main_x", [2, 128]),
                        ("ro1x", [1, 64]), ("ro1h", [17, 64]),
                        ("ro2h1", [17, 64]), ("ro2h2", [16, 64]),
                        ("ro_fc1", [17, 8]), ("ro_fc2", [9, 8])]:
        w_d[name] = nc.dram_tensor(name, shape, F32, kind="ExternalInput")
    out_d = nc.dram_tensor("out", [fut + 1, bc], F32, kind="ExternalOutput")

    with tile.TileContext(nc) as tc:
        consts = tc.alloc_tile_pool(name="consts", bufs=1)
        states = tc.alloc_tile_pool(name="states", bufs=1)
        work = tc.alloc_tile_pool(name="work", bufs=3)
        xst = tc.alloc_tile_pool(name="xst", bufs=8)
        # main psum (4 banks) and rollout psum (4 banks) stay disjoint for the
        # whole kernel: recycling banks across pools while late main-loop ACT
        # reads are in flight corrupts results (PE-write/engine-read same-bank
        # hazard)
        psum = tc.alloc_tile_pool(name="psum", bufs=2, space="PSUM")
        psro = tc.alloc_tile_pool(name="psro", bufs=1, space="PSUM")

        w_sb = {}
        for name, t_d in w_d.items():
            w_sb[name] = consts.tile(list(t_d.shape), F32, tag=name, name=name)
            nc.sync.dma_start(out=w_sb[name], in_=t_d[:])

        # rotating rhs buffers: [h1(0:16); h2(16:32)]
        hx = []
        for q in range(NHX):
            hq = states.tile([32, bc], F32, tag=f"hx{q}", name=f"hx{q}")
            nc.vector.memset(hq, 0.0)
            hx.append(hq)
        cc = states.tile([32, bc], F32, tag="cc")   # [c1; c2]
        nc.vector.memset(cc, 0.0)

        wmh, wmx = w_sb["main_h"], w_sb["main_x"]

        # rollout state tiles (declared early; layer-1 snapshots are taken
        # between main-loop steps T-1 and T)
        h1e = states.tile([17, bc], F32, tag="h1e")  # h1 | ones
        h2e = states.tile([17, bc], F32, tag="h2e")  # h2 | ones
        rc1 = states.tile([16, bc], F32, tag="rc1")
        rc2 = states.tile([16, bc], F32, tag="rc2")

        # ---------------- main teacher-forced loop ----------------
        def body(j):
            cur = hx[j % NHX]
            nxt = hx[(j + 1) % NHX]
            xs = xst.tile([2, bc], F32, tag="xs")
            nc.sync.dma_start(out=xs, in_=xe_d[j])

            g = psum.tile([32, 4, bc], F32, tag="g")  # free: gate-type x batch
            for k in range(4):
                # x+bias then h, closing each accumulation group before the
                # next opens (concurrent groups in one psum zero region are
                # illegal)
                nc.tensor.matmul(g[:, k, :], wmx[:, 32 * k:32 * k + 32], xs,
                                 start=True, stop=False)
                nc.tensor.matmul(g[:, k, :], wmh[:, 32 * k:32 * k + 32], cur,
                                 start=False, stop=True)

            sif = work.tile([32, 3, bc], F32, tag="sif")
            nc.scalar.activation(sif, g[:, 0:3, :], AF.Sigmoid)
            tg = work.tile([32, bc], F32, tag="tg")
            nc.scalar.activation(tg, g[:, 3, :], AF.Tanh)

            # j==0: layer-1 half only (layer-2 gates are not yet valid).
            # j==t_steps: full window (base-16 slices are illegal); the
            # layer-1 results of this step are junk but harmless — rc1/h1e
            # snapshot c1(T-1)/h1(T-1) before this step's writes land.
            s0, s1 = (0, 16) if j == 0 else (0, 32)
            m1 = work.tile([32, bc], F32, tag="m1")
            m2 = work.tile([32, bc], F32, tag="m2")
            tc_ = work.tile([32, bc], F32, tag="tc")
            nc.vector.tensor_mul(m1[s0:s1], sif[s0:s1, 1, :], cc[s0:s1])
            nc.vector.tensor_mul(m2[s0:s1], sif[s0:s1, 0, :], tg[s0:s1])
            nc.vector.tensor_add(cc[s0:s1], m1[s0:s1], m2[s0:s1])
            nc.scalar.activation(tc_[s0:s1], cc[s0:s1], AF.Tanh)
            nc.vector.tensor_mul(nxt[s0:s1], sif[s0:s1, 2, :], tc_[s0:s1])

        n_loop = t_steps if loop_steps is None else loop_steps
        for j in range(n_loop):
            body(j)
        # snapshot layer-1 state before the final (layer-2-only) step clobbers it
        nc.scalar.copy(h1e[0:16], hx[n_loop % NHX][0:16])   # h1(T-1)
        nc.scalar.copy(rc1, cc[0:16])                        # c1(T-1)
        body(n_loop)

        # ---------------- rollout ----------------
        ot = states.tile([1, bc], F32, tag="ot")     # current head output
        zt = states.tile([9, bc], F32, tag="zt")     # leaky(fc1) | ones
        out_sb = states.tile([1, fut + 1, bc], F32, tag="out_sb")
        # ones rows (memset can't start at partition 16/8 — DMA from xe ones row)
        nc.sync.dma_start(out=h1e[16:17, :], in_=xe_d[n_loop, 1:2])
        nc.sync.dma_start(out=h2e[16:17, :], in_=xe_d[n_loop, 1:2])
        nc.sync.dma_start(out=zt[8:9, :], in_=xe_d[n_loop, 1:2])

        nc.sync.dma_start(out=h2e[0:16, :], in_=hx[(n_loop + 1) % NHX][16:32, :])  # h2(T-1), repartition
        nc.sync.dma_start(out=rc2[:], in_=cc[16:32, :])

        if dbg == 1:
            for name, t in [("d_h1e", h1e), ("d_h2e", h2e), ("d_rc1", rc1), ("d_rc2", rc2)]:
                nc.sync.dma_start(out=dbg_d[name][:], in_=t[:])

        last_ops = []

        def head(r):
            z = psro.tile([8, bc], F32, tag="roz")
            nc.tensor.matmul(z, w_sb["ro_fc1"], h2e, start=True, stop=True)
            zs = work.tile([8, bc], F32, tag="zs")
            nc.scalar.mul(zs, z, 0.2)
            nc.vector.tensor_max(zt[0:8], z, zs)  # leaky relu 0.2
            o_ps = psro.tile([8, bc], F32, tag="roo")
            nc.tensor.matmul(o_ps, w_sb["ro_fc2"], zt, start=True, stop=True)
            last_ops[:] = [o_ps]
            nc.scalar.copy(out_sb[:, r, :], o_ps[0:1])
            if r <= fut - 1:
                nc.scalar.copy(ot, o_ps[0:1])

        def ro_cell(mms, rc, h_out):
            gr = psro.tile([16, 4, bc], F32, tag="rog")
            for k in range(4):
                for i, (lhsT, rhs) in enumerate(mms):
                    nc.tensor.matmul(gr[:, k, :], lhsT[:, 16 * k:16 * k + 16], rhs,
                                     start=(i == 0), stop=(i == len(mms) - 1))
            sifr = work.tile([16, 3, bc], F32, tag="sifr")
            nc.scalar.activation(sifr, gr[:, 0:3, :], AF.Sigmoid)
            tgr = work.tile([16, bc], F32, tag="tgr")
            nc.scalar.activation(tgr, gr[:, 3, :], AF.Tanh)
            a1 = work.tile([16, bc], F32, tag="a1")
            a2 = work.tile([16, bc], F32, tag="a2")
            tcr = work.tile([16, bc], F32, tag="tcr")
            nc.vector.tensor_mul(a1, sifr[:, 1, :], rc)
            nc.vector.tensor_mul(a2, sifr[:, 0, :], tgr)
            nc.vector.tensor_add(rc, a1, a2)
            nc.scalar.activation(tcr, rc, AF.Tanh)
            nc.vector.tensor_mul(h_out, sifr[:, 2, :], tcr)

        head(0)
        for r in range(fut):
            ro_cell([(w_sb["ro1x"], ot), (w_sb["ro1h"], h1e)], rc1, h1e[0:16])
            ro_cell([(w_sb["ro2h1"], h1e), (w_sb["ro2h2"], h2e[0:16])], rc2, h2e[0:16])
            head(r + 1)

        if dbg == 2:
            for name, t in [("d_h1e", h1e), ("d_h2e", h2e), ("d_rc1", rc1), ("d_rc2", rc2)]:
                nc.sync.dma_start(out=dbg_d[name][:], in_=t[:])
        if dbg == 3:
            nc.sync.dma_start(out=dbg_d["d_zt"][:], in_=zt[:])
            ops_sb = states.tile([8, bc], F32, tag="ops_sb")
            nc.scalar.copy(ops_sb, last_ops[0])
            nc.sync.dma_start(out=dbg_d["d_ops"][:], in_=ops_sb[:])
            nc.sync.dma_start(out=dbg_d["d_osb"][:], in_=out_sb[:])
            z2 = psro.tile([8, bc], F32, tag="roz")
            nc.tensor.matmul(z2, w_sb["ro_fc1"], h2e, start=True, stop=True)
            z2s = states.tile([8, bc], F32, tag="z2s")
            nc.scalar.copy(z2s, z2)
            nc.sync.dma_start(out=dbg_d["d_z"][:], in_=z2s[:])

        # keep the partition dim in the AP — integer-indexing it away breaks
        # Tile's subtile dependency tracking (the DMA then reads stale data)
        nc.sync.dma_start(out=out_d[:].rearrange("(o f) b -> o f b", o=1), in_=out_sb)

        for p_ in (psro, psum, xst, work, states, consts):
            p_.release()

    if not nc.is_finalized():
        nc.finalize()
    return nc


_CACHED = {}


def _get_nc(t_steps, fut, bc, loop_steps=None):
    key = (t_steps, fut, bc, loop_steps)
    if key not in _CACHED:
        _CACHED[key] = _build(t_steps, fut, bc, loop_steps=loop_steps)
    return _CACHED[key]


def kernel(x, W_ih1, W_hh1, b_ih1, b_hh1, W_ih2, W_hh2, b_ih2, b_hh2,
           fc1_w, fc1_b, fc2_w, fc2_b, future, _t_steps=None, _trace=False,
           _loop_steps=None):
    x = np.asarray(x, np.float32)
    fut = int(future)
    t_steps = int(_t_steps or x.shape[1])
    bc = x.shape[0] // NCORES

    w = _pack_weights(np.asarray(W_ih1, np.float32), np.asarray(W_hh1, np.float32),
                      np.asarray(b_ih1, np.float32), np.asarray(b_hh1, np.float32),
                      np.asarray(W_ih2, np.float32), np.asarray(W_hh2, np.float32),
                      np.asarray(b_ih2, np.float32), np.asarray(b_hh2, np.float32),
                      np.asarray(fc1_w, np.float32), np.asarray(fc1_b, np.float32),
                      np.asarray(fc2_w, np.float32), np.asarray(fc2_b, np.float32))

    nc = _get_nc(t_steps, fut, bc, _loop_steps)
    in_maps = []
    for c in range(NCORES):
        m = dict(w)
        m["xe"] = _pack_x(x[c * bc : (c + 1) * bc, :t_steps], t_steps)
        in_maps.append(m)

    res = run_bass_kernel_spmd(nc, in_maps, core_ids=list(range(NCORES)), trace=_trace)
    outs = [res.results[c]["out"] for c in range(NCORES)]  # each [fut+1, bc]
    full = np.concatenate(outs, axis=1).T  # [B, fut+1]
    kernel._last_exec_ns = res.exec_time_ns
    return np.ascontiguousarray(full.astype(np.float32))



# revision 24
# speedup vs baseline: 196.3174x; 196.3174x over previous
"""Trainium2 Bass kernel for 2-layer LSTM (H=16) time-series predictor.

Model (reference): x:[B,T] -> per-t scalar input into LSTMCell1(1->16) ->
LSTMCell2(16->16), teacher-forced over T steps, then head(h2)=fc2(leaky(fc1(h2)))
produces out[:,0]; 32 autoregressive steps feed head output back as input.
Output [B, 33].

Sharding: data-parallel over batch across 8 cores (256 batch each), weights
replicated. Per-core layout: hidden dim on partitions, batch on the free dim.

Key structure (v2): the teacher-forced recurrence runs inside a hardware
For_i loop (2 steps per iteration for ping-pong buffer rotation) instead of
a 2048x unrolled instruction stream — per-instruction dispatch overhead made
the unrolled version ~100x slower than the loop.  Per step: one DMA loads
[x_t; 1] into partitions 32:34 of the rhs buffer, 4 matmuls (one per gate
type, K=34 folding h1/h2/x/bias), then sigmoid/tanh + 4 DVE ops shared by
both layers (layer 2 lags one step so both layers use one rhs).

Hardware constraints that shape the design:
  * every vector/scalar instruction needs ALL operands on the same partition
    range -> every elementwise tensor (c, h, sigmoid/tanh outputs) lives on
    partition window 0:32 ([layer1; layer2] stacked), and the 4 gate types are
    separated along the PSUM *free* dim instead of partitions;
  * a matmul writes one [M<=32-aligned, N<=512] PSUM block -> 4 matmuls per
    step, one per gate type (i, f, o, g), each [32, 256].
"""

import numpy as np

import concourse.bass as bass
import concourse.tile as tile
from concourse import bacc, mybir
from concourse.bass import ds
from concourse.bass_utils import run_bass_kernel_spmd

F32 = mybir.dt.float32
AF = mybir.ActivationFunctionType

H = 16
B = 2048
T = 2048
FUT = 32
NCORES = 8
BC = B // NCORES  # 256 batch per core

# torch gate row order in the 4H weight matrices: i, f, g, o
_G = {"i": slice(0, H), "f": slice(H, 2 * H), "g": slice(2 * H, 3 * H), "o": slice(3 * H, 4 * H)}
# our gate order along the psum free dim / lhsT column blocks
_ORDER = ["i", "f", "o", "g"]


def _pack_weights(W_ih1, W_hh1, b_ih1, b_hh1, W_ih2, W_hh2, b_ih2, b_hh2,
                  fc1_w, fc1_b, fc2_w, fc2_b):
    b1 = b_ih1 + b_hh1  # [64]
    b2 = b_ih2 + b_hh2

    # main loop lhsT; column block k (32 wide) = gate _ORDER[k], [l1(16)|l2(16)].
    # rows = [h1(16); h2(16); x(1); ones(1)].
    main_hx = np.zeros((34, 128), np.float32)
    for k, gn in enumerate(_ORDER):
        c0 = 32 * k
        main_hx[0:16, c0:c0 + 16] = W_hh1[_G[gn], :].T       # h1 -> layer1 gate
        main_hx[0:16, c0 + 16:c0 + 32] = W_ih2[_G[gn], :].T  # h1 -> layer2 gate
        main_hx[16:32, c0 + 16:c0 + 32] = W_hh2[_G[gn], :].T # h2 -> layer2 gate
        main_hx[32, c0:c0 + 16] = W_ih1[_G[gn], 0]           # x  -> layer1 gate
        main_hx[33, c0:c0 + 16] = b1[_G[gn]]
        main_hx[33, c0 + 16:c0 + 32] = b2[_G[gn]]

    # rollout layer1: split into K=1 part (input o) and K=17 part (h1 + bias)
    ro1x = np.zeros((1, 64), np.float32)   # columns: 4 gate blocks of 16
    ro1h = np.zeros((17, 64), np.float32)
    ro2h1 = np.zeros((17, 64), np.float32)  # h1 + bias part of layer2
    ro2h2 = np.zeros((16, 64), np.float32)  # h2 part of layer2
    for k, gn in enumerate(_ORDER):
        c0 = 16 * k
        ro1x[0, c0:c0 + 16] = W_ih1[_G[gn], 0]
        ro1h[0:16, c0:c0 + 16] = W_hh1[_G[gn], :].T
        ro1h[16, c0:c0 + 16] = b1[_G[gn]]
        ro2h1[0:16, c0:c0 + 16] = W_ih2[_G[gn], :].T
        ro2h1[16, c0:c0 + 16] = b2[_G[gn]]
        ro2h2[0:16, c0:c0 + 16] = W_hh2[_G[gn], :].T

    ro_fc1 = np.zeros((17, 8), np.float32)
    ro_fc1[0:16] = fc1_w.T  # fc1_w [8,16]
    ro_fc1[16] = fc1_b

    # M=1 matmuls misbehave on HW — replicate the fc2 column into M=8 and
    # read row 0 of the result instead
    ro_fc2 = np.zeros((9, 8), np.float32)
    ro_fc2[0:8] = fc2_w.T  # fc2_w [1,8]
    ro_fc2[8] = fc2_b

    return dict(main_hx=main_hx, ro1x=ro1x, ro1h=ro1h,
                ro2h1=ro2h1, ro2h2=ro2h2, ro_fc1=ro_fc1, ro_fc2=ro_fc2)


def _pack_x(x_core, t_steps):
    """x_core [BC, t] -> [t+3, BC]:
    rows 0:2 ones; rows 2:5 = x_0, x_1, x_2 (prologue); rows 5+2p, 6+2p =
    [x_{2p+4}, x_{2p+3}] (slot-0/slot-1 prefetch pair for loop pair p).
    x_t for t >= t_steps reads as 0."""
    bcn = x_core.shape[0]
    xv = np.zeros((t_steps + 1, bcn), np.float32)  # x_0 .. x_{t_steps} (last 0)
    xv[:t_steps] = x_core.T.astype(np.float32)
    xe = np.empty((t_steps + 3, bcn), np.float32)
    xe[0:2] = 1.0
    xe[2:5] = xv[0:3]
    npairs = (t_steps - 2) // 2
    idx = np.empty(2 * npairs, np.int64)
    idx[0::2] = 4 + 2 * np.arange(npairs)  # x_{2p+4} -> slot 0
    idx[1::2] = 3 + 2 * np.arange(npairs)  # x_{2p+3} -> slot 1
    xe[5:5 + 2 * npairs] = xv[idx]
    return xe


# ---------------------------------------------------------------------------
# device kernel
# ---------------------------------------------------------------------------

def _build(t_steps=T, fut=FUT, bc=BC, loop_steps=None, _outer=1, _u2=1,
           _stag=True, _dmode="split_early", _pool_m1=False):
    # loop_steps < t_steps runs fewer recurrence steps with identical I/O
    # sizes — for isolating device time via wall-clock deltas (output is
    # mathematically meaningless in that mode)
    n_loop = t_steps if loop_steps is None else loop_steps
    assert n_loop % 2 == 0 and n_loop >= 4
    n_it = (n_loop - 2) // 2  # For_i iterations, covering j = 1 .. n_loop-2

    nc = bacc.Bacc("TRN2", target_bir_lowering=False)

    xe_d = nc.dram_tensor("xe", [t_steps + 3, bc], F32, kind="ExternalInput")
    w_d = {}
    for name, shape in [("main_hx", [34, 128]),
                        ("ro1x", [1, 64]), ("ro1h", [17, 64]),
                        ("ro2h1", [17, 64]), ("ro2h2", [16, 64]),
                        ("ro_fc1", [17, 8]), ("ro_fc2", [9, 8])]:
        w_d[name] = nc.dram_tensor(name, shape, F32, kind="ExternalInput")
    out_d = nc.dram_tensor("out", [fut + 1, bc], F32, kind="ExternalOutput")

    with tile.TileContext(nc) as tc:
        consts = tc.alloc_tile_pool(name="consts", bufs=1)
        states = tc.alloc_tile_pool(name="states", bufs=1)
        work = tc.alloc_tile_pool(name="work", bufs=1)
        # main psum (4 banks) and rollout psum (4 banks) stay disjoint for the
        # whole kernel: recycling banks across pools while late main-loop ACT
        # reads are in flight corrupts results (PE-write/engine-read same-bank
        # hazard)
        psum = tc.alloc_tile_pool(name="psum", bufs=1, space="PSUM")
        psro = tc.alloc_tile_pool(name="psro", bufs=1, space="PSUM")

        w_sb = {}
        for name, t_d in w_d.items():
            w_sb[name] = consts.tile(list(t_d.shape), F32, tag=name, name=name)
            nc.sync.dma_start(out=w_sb[name], in_=t_d[:])

        # ping-pong rhs buffers as free-dim slots of one tile so one DMA can
        # prefetch both slots' x rows: [h1(0:16); h2(16:32); x(32); ones(33)]
        hxT = states.tile([34, 2, bc], F32, tag="hxT", name="hxT")
        nc.vector.memset(hxT, 0.0)
        hx = [hxT[:, 0, :], hxT[:, 1, :]]
        cc = states.tile([32, bc], F32, tag="cc")   # [c1; c2]
        nc.vector.memset(cc, 0.0)

        whx = w_sb["main_hx"]

        # rollout state tiles (declared early; layer-1 snapshots are taken
        # between main-loop steps n_loop-1 and n_loop)
        h1e = states.tile([17, bc], F32, tag="h1e")  # h1 | ones
        h2e = states.tile([17, bc], F32, tag="h2e")  # h2 | ones
        rc1 = states.tile([16, bc], F32, tag="rc1")
        rc2 = states.tile([16, bc], F32, tag="rc2")

        # ---------------- main teacher-forced loop ----------------
        def body(cur, nxt, u, first=False):
            # first step: layer-1 half only (layer-2 state must stay zero)
            p1 = 16 if first else 32
            g = psum.tile([32, 4, bc], F32, tag=f"g{u & 1}", name=f"g{u & 1}")
            for k in range(4):
                nc.tensor.matmul(g[:, k, :], whx[:, 32 * k:32 * k + 32], cur,
                                 start=True, stop=True)
            sif = work.tile([32, 3, bc], F32, tag=f"sif{u}", name="sif")
            nc.scalar.activation(sif[0:p1], g[0:p1, 0:3, :], AF.Sigmoid)
            tg = work.tile([32, bc], F32, tag=f"tg{u}", name="tg")
            nc.scalar.activation(tg[0:p1], g[0:p1, 3, :], AF.Tanh)
            m1 = work.tile([32, bc], F32, tag=f"m1{u}", name="m1")
            m2 = work.tile([32, bc], F32, tag=f"m2{u}", name="m2")
            tc_ = work.tile([32, bc], F32, tag=f"tc{u}", name="tc_")
            mul1 = nc.gpsimd.tensor_mul if _pool_m1 else nc.vector.tensor_mul
            mul1(m1[0:p1], sif[0:p1, 1, :], cc[0:p1])
            nc.vector.tensor_mul(m2[0:p1], sif[0:p1, 0, :], tg[0:p1])
            nc.vector.tensor_add(cc[0:p1], m1[0:p1], m2[0:p1])
            nc.scalar.activation(tc_[0:p1], cc[0:p1], AF.Tanh)
            nc.vector.tensor_mul(nxt[0:p1], sif[0:p1, 2, :], tc_[0:p1])

        # prologue: ones rows once, x_0 -> slot0; step j=0 (layer-1 only);
        # x_1 -> slot1, x_2 -> slot0
        nc.sync.dma_start(out=hxT[33:34, 0:2, :], in_=xe_d[0:2])
        nc.sync.dma_start(out=hxT[32:33, 0:1, :], in_=xe_d[2:3])
        body(hx[0], hx[1], u=0, first=True)
        nc.sync.dma_start(out=hxT[32:33, 1:2, :], in_=xe_d[3:4])
        nc.sync.dma_start(out=hxT[32:33, 0:1, :], in_=xe_d[4:5])

        # iterations cover steps j = 2i+1 (slot1 -> slot0), j = 2i+2
        # (slot0 -> slot1), then prefetch x_{2i+3} / x_{2i+4}
        # pair(p) = steps j=2p+1 (slot1->slot0) and j=2p+2 (slot0->slot1),
        # prefetching x_{2p+4} -> slot0, x_{2p+3} -> slot1 (xe rows 5+2p, 6+2p)
        def xrow(r):
            return xe_d[r:r + 1] if isinstance(r, int) else xe_d[ds(r, 1)]

        def pair(p_base, q):
            p2 = (p_base + q) * 2
            if _dmode == "merged":
                body(hx[1], hx[0], u=2 * q)
                body(hx[0], hx[1], u=2 * q + 1)
                in_ = (xe_d[p2 + 5:p2 + 7] if isinstance(p_base, int)
                       else xe_d[ds(p2 + 5, 2)])
                nc.sync.dma_start(out=hxT[32:33, 0:2, :], in_=in_)
            elif _dmode == "split_early":
                body(hx[1], hx[0], u=2 * q)
                nc.sync.dma_start(out=hxT[32:33, 1:2, :], in_=xrow(p2 + 6))
                body(hx[0], hx[1], u=2 * q + 1)
                nc.sync.dma_start(out=hxT[32:33, 0:1, :], in_=xrow(p2 + 5))
            else:  # split_end
                body(hx[1], hx[0], u=2 * q)
                body(hx[0], hx[1], u=2 * q + 1)
                nc.sync.dma_start(out=hxT[32:33, 1:2, :], in_=xrow(p2 + 6))
                nc.sync.dma_start(out=hxT[32:33, 0:1, :], in_=xrow(p2 + 5))

        u2 = _u2
        n_pairs = n_it  # (n_loop-2)//2 pairs in total
        n_lit = n_pairs // u2       # For_i trip count
        n_tail = n_pairs - n_lit * u2  # statically peeled tail pairs

        def main_loop():
            with tc.For_i(0, n_lit, 1, staggered_reset=_stag) as ip:
                for q in range(u2):
                    pair(ip * u2, q)

        if _outer == 1:
            main_loop()
        else:
            with tc.For_i(0, _outer, 1) as _rep:
                main_loop()
        for tp in range(n_tail):
            pair(n_lit * u2, tp)

        # epilogue: step j = n_loop-1, snapshot layer-1 state, then the final
        # (layer-2-only) step j = n_loop whose layer-1 results are junk
        body(hx[1], hx[0], u=0)
        nc.scalar.copy(h1e[0:16], hx[0][0:16])   # h1(T-1)
        nc.scalar.copy(rc1, cc[0:16])            # c1(T-1)
        body(hx[0], hx[1], u=1)

        # ---------------- rollout ----------------
        ot = states.tile([1, bc], F32, tag="ot")     # current head output
        zt = states.tile([9, bc], F32, tag="zt")     # leaky(fc1) | ones
        out_sb = states.tile([1, fut + 1, bc], F32, tag="out_sb")
        # ones rows (memset can't start at partition 16/8 — DMA from xe ones row)
        nc.sync.dma_start(out=h1e[16:17, :], in_=xe_d[1:2])
        nc.sync.dma_start(out=h2e[16:17, :], in_=xe_d[1:2])
        nc.sync.dma_start(out=zt[8:9, :], in_=xe_d[1:2])

        nc.sync.dma_start(out=h2e[0:16, :], in_=hx[1][16:32, :])  # h2(T-1), repartition
        nc.sync.dma_start(out=rc2[:], in_=cc[16:32, :])

        wkro = tc.alloc_tile_pool(name="wkro", bufs=3)

        def head(r):
            z = psro.tile([8, bc], F32, tag="roz")
            nc.tensor.matmul(z, w_sb["ro_fc1"], h2e, start=True, stop=True)
            zs = wkro.tile([8, bc], F32, tag="zs")
            nc.scalar.mul(zs, z, 0.2)
            nc.vector.tensor_max(zt[0:8], z, zs)  # leaky relu 0.2
            o_ps = psro.tile([8, bc], F32, tag="roo")
            nc.tensor.matmul(o_ps, w_sb["ro_fc2"], zt, start=True, stop=True)
            nc.scalar.copy(out_sb[:, r, :], o_ps[0:1])
            if r <= fut - 1:
                nc.scalar.copy(ot, o_ps[0:1])

        def ro_cell(mms, rc, h_out):
            gr = psro.tile([16, 4, bc], F32, tag="rog")
            for k in range(4):
                for i, (lhsT, rhs) in enumerate(mms):
                    nc.tensor.matmul(gr[:, k, :], lhsT[:, 16 * k:16 * k + 16], rhs,
                                     start=(i == 0), stop=(i == len(mms) - 1))
            sifr = wkro.tile([16, 3, bc], F32, tag="sifr")
            nc.scalar.activation(sifr, gr[:, 0:3, :], AF.Sigmoid)
            tgr = wkro.tile([16, bc], F32, tag="tgr")
            nc.scalar.activation(tgr, gr[:, 3, :], AF.Tanh)
            a1 = wkro.tile([16, bc], F32, tag="a1")
            a2 = wkro.tile([16, bc], F32, tag="a2")
            tcr = wkro.tile([16, bc], F32, tag="tcr")
            nc.vector.tensor_mul(a1, sifr[:, 1, :], rc)
            nc.vector.tensor_mul(a2, sifr[:, 0, :], tgr)
            nc.vector.tensor_add(rc, a1, a2)
            nc.scalar.activation(tcr, rc, AF.Tanh)
            nc.vector.tensor_mul(h_out, sifr[:, 2, :], tcr)

        head(0)
        for r in range(fut):
            ro_cell([(w_sb["ro1x"], ot), (w_sb["ro1h"], h1e)], rc1, h1e[0:16])
            ro_cell([(w_sb["ro2h1"], h1e), (w_sb["ro2h2"], h2e[0:16])], rc2, h2e[0:16])
            head(r + 1)

        # keep the partition dim in the AP — integer-indexing it away breaks
        # Tile's subtile dependency tracking (the DMA then reads stale data)
        nc.sync.dma_start(out=out_d[:].rearrange("(o f) b -> o f b", o=1), in_=out_sb)

        for p_ in (wkro, psro, psum, work, states, consts):
            p_.release()

    if not nc.is_finalized():
        nc.finalize()
    return nc


_CACHED = {}


def _get_nc(t_steps, fut, bc, loop_steps=None, outer=1):
    key = (t_steps, fut, bc, loop_steps, outer)
    if key not in _CACHED:
        _CACHED[key] = _build(t_steps, fut, bc, loop_steps=loop_steps,
                              _outer=outer)
    return _CACHED[key]


def kernel(x, W_ih1, W_hh1, b_ih1, b_hh1, W_ih2, W_hh2, b_ih2, b_hh2,
           fc1_w, fc1_b, fc2_w, fc2_b, future, _t_steps=None, _trace=False,
           _loop_steps=None, _outer=1):
    x = np.asarray(x, np.float32)
    fut = int(future)
    t_steps = int(_t_steps or x.shape[1])
    bc = x.shape[0] // NCORES

    w = _pack_weights(np.asarray(W_ih1, np.float32), np.asarray(W_hh1, np.float32),
                      np.asarray(b_ih1, np.float32), np.asarray(b_hh1, np.float32),
                      np.asarray(W_ih2, np.float32), np.asarray(W_hh2, np.float32),
                      np.asarray(b_ih2, np.float32), np.asarray(b_hh2, np.float32),
                      np.asarray(fc1_w, np.float32), np.asarray(fc1_b, np.float32),
                      np.asarray(fc2_w, np.float32), np.asarray(fc2_b, np.float32))

    nc = _get_nc(t_steps, fut, bc, _loop_steps, int(_outer))
    in_maps = []
    for c in range(NCORES):
        m = dict(w)
        m["xe"] = _pack_x(x[c * bc : (c + 1) * bc, :t_steps], t_steps)
        in_maps.append(m)

    res = run_bass_kernel_spmd(nc, in_maps, core_ids=list(range(NCORES)), trace=_trace)
    outs = [res.results[c]["out"] for c in range(NCORES)]  # each [fut+1, bc]
    full = np.concatenate(outs, axis=1).T  # [B, fut+1]
    kernel._last_exec_ns = res.exec_time_ns
    return np.ascontiguousarray(full.astype(np.float32))


# revision 26
# speedup vs baseline: 238.1082x; 1.2129x over previous
"""Trainium2 Bass kernel for 2-layer LSTM (H=16) time-series predictor.

Model (reference): x:[B,T] -> per-t scalar input into LSTMCell1(1->16) ->
LSTMCell2(16->16), teacher-forced over T steps, then head(h2)=fc2(leaky(fc1(h2)))
produces out[:,0]; 32 autoregressive steps feed head output back as input.
Output [B, 33].

Sharding: data-parallel over batch across 8 cores (256 batch each), weights
replicated. Per-core layout: hidden dim on partitions, batch on the free dim.

Key structure (v2): the teacher-forced recurrence runs inside a hardware
For_i loop (2 steps per iteration for ping-pong buffer rotation) instead of
a 2048x unrolled instruction stream — per-instruction dispatch overhead made
the unrolled version ~100x slower than the loop.  Per step: one DMA loads
[x_t; 1] into partitions 32:34 of the rhs buffer, 4 matmuls (one per gate
type, K=34 folding h1/h2/x/bias), then sigmoid/tanh + 4 DVE ops shared by
both layers (layer 2 lags one step so both layers use one rhs).

Hardware constraints that shape the design:
  * every vector/scalar instruction needs ALL operands on the same partition
    range -> every elementwise tensor (c, h, sigmoid/tanh outputs) lives on
    partition window 0:32 ([layer1; layer2] stacked), and the 4 gate types are
    separated along the PSUM *free* dim instead of partitions;
  * a matmul writes one [M<=32-aligned, N<=512] PSUM block -> 4 matmuls per
    step, one per gate type (i, f, o, g), each [32, 256].
"""

import numpy as np

import concourse.bass as bass
import concourse.tile as tile
from concourse import bacc, mybir
from concourse.bass import ds
from concourse.bass_utils import run_bass_kernel_spmd

F32 = mybir.dt.float32
AF = mybir.ActivationFunctionType

H = 16
B = 2048
T = 2048
FUT = 32
NCORES = 8
BC = B // NCORES  # 256 batch per core

# torch gate row order in the 4H weight matrices: i, f, g, o
_G = {"i": slice(0, H), "f": slice(H, 2 * H), "g": slice(2 * H, 3 * H), "o": slice(3 * H, 4 * H)}
# our gate order along the psum free dim / lhsT column blocks
_ORDER = ["i", "f", "o", "g"]


def _pack_weights(W_ih1, W_hh1, b_ih1, b_hh1, W_ih2, W_hh2, b_ih2, b_hh2,
                  fc1_w, fc1_b, fc2_w, fc2_b):
    b1 = b_ih1 + b_hh1  # [64]
    b2 = b_ih2 + b_hh2

    # main loop lhsT; column block k (32 wide) = gate _ORDER[k], [l1(16)|l2(16)].
    # rows = [h1(16); h2(16); x(1); ones(1)].
    main_hx = np.zeros((34, 128), np.float32)
    for k, gn in enumerate(_ORDER):
        c0 = 32 * k
        main_hx[0:16, c0:c0 + 16] = W_hh1[_G[gn], :].T       # h1 -> layer1 gate
        main_hx[0:16, c0 + 16:c0 + 32] = W_ih2[_G[gn], :].T  # h1 -> layer2 gate
        main_hx[16:32, c0 + 16:c0 + 32] = W_hh2[_G[gn], :].T # h2 -> layer2 gate
        main_hx[32, c0:c0 + 16] = W_ih1[_G[gn], 0]           # x  -> layer1 gate
        main_hx[33, c0:c0 + 16] = b1[_G[gn]]
        main_hx[33, c0 + 16:c0 + 32] = b2[_G[gn]]

    # rollout layer1: split into K=1 part (input o) and K=17 part (h1 + bias)
    ro1x = np.zeros((1, 64), np.float32)   # columns: 4 gate blocks of 16
    ro1h = np.zeros((17, 64), np.float32)
    ro2h1 = np.zeros((17, 64), np.float32)  # h1 + bias part of layer2
    ro2h2 = np.zeros((16, 64), np.float32)  # h2 part of layer2
    for k, gn in enumerate(_ORDER):
        c0 = 16 * k
        ro1x[0, c0:c0 + 16] = W_ih1[_G[gn], 0]
        ro1h[0:16, c0:c0 + 16] = W_hh1[_G[gn], :].T
        ro1h[16, c0:c0 + 16] = b1[_G[gn]]
        ro2h1[0:16, c0:c0 + 16] = W_ih2[_G[gn], :].T
        ro2h1[16, c0:c0 + 16] = b2[_G[gn]]
        ro2h2[0:16, c0:c0 + 16] = W_hh2[_G[gn], :].T

    ro_fc1 = np.zeros((17, 8), np.float32)
    ro_fc1[0:16] = fc1_w.T  # fc1_w [8,16]
    ro_fc1[16] = fc1_b

    # M=1 matmuls misbehave on HW — replicate the fc2 column into M=8 and
    # read row 0 of the result instead
    ro_fc2 = np.zeros((9, 8), np.float32)
    ro_fc2[0:8] = fc2_w.T  # fc2_w [1,8]
    ro_fc2[8] = fc2_b

    return dict(main_hx=main_hx, ro1x=ro1x, ro1h=ro1h,
                ro2h1=ro2h1, ro2h2=ro2h2, ro_fc1=ro_fc1, ro_fc2=ro_fc2)


def _pack_x(x_core, t_steps):
    """x_core [BC, t] -> [t+3, BC]:
    rows 0:2 ones; rows 2:5 = x_0, x_1, x_2 (prologue); rows 5+2p, 6+2p =
    [x_{2p+4}, x_{2p+3}] (slot-0/slot-1 prefetch pair for loop pair p).
    x_t for t >= t_steps reads as 0."""
    bcn = x_core.shape[0]
    xv = np.zeros((t_steps + 1, bcn), np.float32)  # x_0 .. x_{t_steps} (last 0)
    xv[:t_steps] = x_core.T.astype(np.float32)
    xe = np.empty((t_steps + 3, bcn), np.float32)
    xe[0:2] = 1.0
    xe[2:5] = xv[0:3]
    npairs = (t_steps - 2) // 2
    idx = np.empty(2 * npairs, np.int64)
    idx[0::2] = 4 + 2 * np.arange(npairs)  # x_{2p+4} -> slot 0
    idx[1::2] = 3 + 2 * np.arange(npairs)  # x_{2p+3} -> slot 1
    xe[5:5 + 2 * npairs] = xv[idx]
    return xe


# ---------------------------------------------------------------------------
# device kernel
# ---------------------------------------------------------------------------

def _build(t_steps=T, fut=FUT, bc=BC, loop_steps=None, _outer=1, _u2=1,
           _stag=True, _dmode="split_early", _pool_m1=False):
    # loop_steps < t_steps runs fewer recurrence steps with identical I/O
    # sizes — for isolating device time via wall-clock deltas (output is
    # mathematically meaningless in that mode)
    n_loop = t_steps if loop_steps is None else loop_steps
    assert n_loop % 2 == 0 and n_loop >= 4
    n_it = (n_loop - 2) // 2  # For_i iterations, covering j = 1 .. n_loop-2

    nc = bacc.Bacc("TRN2", target_bir_lowering=False)

    xe_d = nc.dram_tensor("xe", [t_steps + 3, bc], F32, kind="ExternalInput")
    w_d = {}
    for name, shape in [("main_hx", [34, 128]),
                        ("ro1x", [1, 64]), ("ro1h", [17, 64]),
                        ("ro2h1", [17, 64]), ("ro2h2", [16, 64]),
                        ("ro_fc1", [17, 8]), ("ro_fc2", [9, 8])]:
        w_d[name] = nc.dram_tensor(name, shape, F32, kind="ExternalInput")
    out_d = nc.dram_tensor("out", [fut + 1, bc], F32, kind="ExternalOutput")

    with tile.TileContext(nc) as tc:
        consts = tc.alloc_tile_pool(name="consts", bufs=1)
        states = tc.alloc_tile_pool(name="states", bufs=1)
        work = tc.alloc_tile_pool(name="work", bufs=1)
        # main psum (4 banks) and rollout psum (4 banks) stay disjoint for the
        # whole kernel: recycling banks across pools while late main-loop ACT
        # reads are in flight corrupts results (PE-write/engine-read same-bank
        # hazard)
        psum = tc.alloc_tile_pool(name="psum", bufs=1, space="PSUM")
        psro = tc.alloc_tile_pool(name="psro", bufs=1, space="PSUM")

        w_sb = {}
        for name, t_d in w_d.items():
            w_sb[name] = consts.tile(list(t_d.shape), F32, tag=name, name=name)
            nc.sync.dma_start(out=w_sb[name], in_=t_d[:])

        # ping-pong rhs buffers as free-dim slots of one tile so one DMA can
        # prefetch both slots' x rows: [h1(0:16); h2(16:32); x(32); ones(33)]
        hxT = states.tile([34, 2, bc], F32, tag="hxT", name="hxT")
        nc.vector.memset(hxT, 0.0)
        hx = [hxT[:, 0, :], hxT[:, 1, :]]
        cc = states.tile([32, bc], F32, tag="cc")   # [c1; c2]
        nc.vector.memset(cc, 0.0)

        whx = w_sb["main_hx"]

        # rollout state tiles (declared early; layer-1 snapshots are taken
        # between main-loop steps n_loop-1 and n_loop)
        h1e = states.tile([17, bc], F32, tag="h1e")  # h1 | ones
        h2e = states.tile([17, bc], F32, tag="h2e")  # h2 | ones
        rc1 = states.tile([16, bc], F32, tag="rc1")
        rc2 = states.tile([16, bc], F32, tag="rc2")

        # ---------------- main teacher-forced loop ----------------
        def body(cur, nxt, u, first=False):
            # first step: layer-1 half only (layer-2 state must stay zero)
            p1 = 16 if first else 32
            g = psum.tile([32, 4, bc], F32, tag=f"g{u & 1}", name=f"g{u & 1}")
            for k in range(4):
                nc.tensor.matmul(g[:, k, :], whx[:, 32 * k:32 * k + 32], cur,
                                 start=True, stop=True)
            sif = work.tile([32, 3, bc], F32, tag=f"sif{u}", name="sif")
            nc.scalar.activation(sif[0:p1], g[0:p1, 0:3, :], AF.Sigmoid)
            tg = work.tile([32, bc], F32, tag=f"tg{u}", name="tg")
            nc.scalar.activation(tg[0:p1], g[0:p1, 3, :], AF.Tanh)
            m1 = work.tile([32, bc], F32, tag=f"m1{u}", name="m1")
            m2 = work.tile([32, bc], F32, tag=f"m2{u}", name="m2")
            tc_ = work.tile([32, bc], F32, tag=f"tc{u}", name="tc_")
            mul1 = nc.gpsimd.tensor_mul if _pool_m1 else nc.vector.tensor_mul
            mul1(m1[0:p1], sif[0:p1, 1, :], cc[0:p1])
            nc.vector.tensor_mul(m2[0:p1], sif[0:p1, 0, :], tg[0:p1])
            nc.vector.tensor_add(cc[0:p1], m1[0:p1], m2[0:p1])
            nc.scalar.activation(tc_[0:p1], cc[0:p1], AF.Tanh)
            nc.vector.tensor_mul(nxt[0:p1], sif[0:p1, 2, :], tc_[0:p1])

        # prologue: ones rows once, x_0 -> slot0; step j=0 (layer-1 only);
        # x_1 -> slot1, x_2 -> slot0
        nc.sync.dma_start(out=hxT[33:34, 0:2, :], in_=xe_d[0:2])
        nc.sync.dma_start(out=hxT[32:33, 0:1, :], in_=xe_d[2:3])
        body(hx[0], hx[1], u=0, first=True)
        nc.sync.dma_start(out=hxT[32:33, 1:2, :], in_=xe_d[3:4])
        nc.sync.dma_start(out=hxT[32:33, 0:1, :], in_=xe_d[4:5])

        # iterations cover steps j = 2i+1 (slot1 -> slot0), j = 2i+2
        # (slot0 -> slot1), then prefetch x_{2i+3} / x_{2i+4}
        # pair(p) = steps j=2p+1 (slot1->slot0) and j=2p+2 (slot0->slot1),
        # prefetching x_{2p+4} -> slot0, x_{2p+3} -> slot1 (xe rows 5+2p, 6+2p)
        def xrow(r):
            return xe_d[r:r + 1] if isinstance(r, int) else xe_d[ds(r, 1)]

        def pair(p_base, q):
            p2 = (p_base + q) * 2
            if _dmode == "merged":
                body(hx[1], hx[0], u=2 * q)
                body(hx[0], hx[1], u=2 * q + 1)
                in_ = (xe_d[p2 + 5:p2 + 7] if isinstance(p_base, int)
                       else xe_d[ds(p2 + 5, 2)])
                nc.sync.dma_start(out=hxT[32:33, 0:2, :], in_=in_)
            elif _dmode == "split_early":
                body(hx[1], hx[0], u=2 * q)
                nc.sync.dma_start(out=hxT[32:33, 1:2, :], in_=xrow(p2 + 6))
                body(hx[0], hx[1], u=2 * q + 1)
                nc.sync.dma_start(out=hxT[32:33, 0:1, :], in_=xrow(p2 + 5))
            else:  # split_end
                body(hx[1], hx[0], u=2 * q)
                body(hx[0], hx[1], u=2 * q + 1)
                nc.sync.dma_start(out=hxT[32:33, 1:2, :], in_=xrow(p2 + 6))
                nc.sync.dma_start(out=hxT[32:33, 0:1, :], in_=xrow(p2 + 5))

        u2 = _u2
        n_pairs = n_it  # (n_loop-2)//2 pairs in total
        n_lit = n_pairs // u2       # For_i trip count
        n_tail = n_pairs - n_lit * u2  # statically peeled tail pairs

        def main_loop():
            with tc.For_i(0, n_lit, 1, staggered_reset=_stag) as ip:
                for q in range(u2):
                    pair(ip * u2, q)

        if _outer == 1:
            main_loop()
        else:
            with tc.For_i(0, _outer, 1) as _rep:
                main_loop()
        for tp in range(n_tail):
            pair(n_lit * u2, tp)

        # epilogue: step j = n_loop-1, snapshot layer-1 state, then the final
        # (layer-2-only) step j = n_loop whose layer-1 results are junk
        body(hx[1], hx[0], u=0)
        nc.scalar.copy(h1e[0:16], hx[0][0:16])   # h1(T-1)
        nc.scalar.copy(rc1, cc[0:16])            # c1(T-1)
        body(hx[0], hx[1], u=1)

        # ---------------- rollout ----------------
        ot = states.tile([1, bc], F32, tag="ot")     # current head output
        zt = states.tile([9, bc], F32, tag="zt")     # leaky(fc1) | ones
        out_sb = states.tile([1, fut + 1, bc], F32, tag="out_sb")
        # ones rows (memset can't start at partition 16/8 — DMA from xe ones row)
        nc.sync.dma_start(out=h1e[16:17, :], in_=xe_d[1:2])
        nc.sync.dma_start(out=h2e[16:17, :], in_=xe_d[1:2])
        nc.sync.dma_start(out=zt[8:9, :], in_=xe_d[1:2])

        nc.sync.dma_start(out=h2e[0:16, :], in_=hx[1][16:32, :])  # h2(T-1), repartition
        nc.sync.dma_start(out=rc2[:], in_=cc[16:32, :])

        wkro = tc.alloc_tile_pool(name="wkro", bufs=3)

        def head(r):
            z = psro.tile([8, bc], F32, tag="roz")
            nc.tensor.matmul(z, w_sb["ro_fc1"], h2e, start=True, stop=True)
            zs = wkro.tile([8, bc], F32, tag="zs")
            nc.scalar.mul(zs, z, 0.2)
            nc.vector.tensor_max(zt[0:8], z, zs)  # leaky relu 0.2
            o_ps = psro.tile([8, bc], F32, tag="roo")
            nc.tensor.matmul(o_ps, w_sb["ro_fc2"], zt, start=True, stop=True)
            if isinstance(r, int):
                nc.scalar.copy(out_sb[:, r, :], o_ps[0:1])
            else:
                nc.scalar.copy(out_sb[:, ds(r, 1), :], o_ps[0:1])
            nc.scalar.copy(ot, o_ps[0:1])

        def ro_cell(mms, rc, h_out):
            gr = psro.tile([16, 4, bc], F32, tag="rog")
            for k in range(4):
                for i, (lhsT, rhs) in enumerate(mms):
                    nc.tensor.matmul(gr[:, k, :], lhsT[:, 16 * k:16 * k + 16], rhs,
                                     start=(i == 0), stop=(i == len(mms) - 1))
            sifr = wkro.tile([16, 3, bc], F32, tag="sifr")
            nc.scalar.activation(sifr, gr[:, 0:3, :], AF.Sigmoid)
            tgr = wkro.tile([16, bc], F32, tag="tgr")
            nc.scalar.activation(tgr, gr[:, 3, :], AF.Tanh)
            a1 = wkro.tile([16, bc], F32, tag="a1")
            a2 = wkro.tile([16, bc], F32, tag="a2")
            tcr = wkro.tile([16, bc], F32, tag="tcr")
            nc.vector.tensor_mul(a1, sifr[:, 1, :], rc)
            nc.vector.tensor_mul(a2, sifr[:, 0, :], tgr)
            nc.vector.tensor_add(rc, a1, a2)
            nc.scalar.activation(tcr, rc, AF.Tanh)
            nc.vector.tensor_mul(h_out, sifr[:, 2, :], tcr)

        head(0)
        with tc.For_i(0, fut, 1) as rv:
            ro_cell([(w_sb["ro1x"], ot), (w_sb["ro1h"], h1e)], rc1, h1e[0:16])
            ro_cell([(w_sb["ro2h1"], h1e), (w_sb["ro2h2"], h2e[0:16])], rc2,
                    h2e[0:16])
            head(rv + 1)

        # keep the partition dim in the AP — integer-indexing it away breaks
        # Tile's subtile dependency tracking (the DMA then reads stale data)
        nc.sync.dma_start(out=out_d[:].rearrange("(o f) b -> o f b", o=1), in_=out_sb)

        for p_ in (wkro, psro, psum, work, states, consts):
            p_.release()

    if not nc.is_finalized():
        nc.finalize()
    return nc


_CACHED = {}


def _get_nc(t_steps, fut, bc, loop_steps=None, outer=1):
    key = (t_steps, fut, bc, loop_steps, outer)
    if key not in _CACHED:
        _CACHED[key] = _build(t_steps, fut, bc, loop_steps=loop_steps,
                              _outer=outer)
    return _CACHED[key]


def kernel(x, W_ih1, W_hh1, b_ih1, b_hh1, W_ih2, W_hh2, b_ih2, b_hh2,
           fc1_w, fc1_b, fc2_w, fc2_b, future, _t_steps=None, _trace=False,
           _loop_steps=None, _outer=1):
    x = np.asarray(x, np.float32)
    fut = int(future)
    t_steps = int(_t_steps or x.shape[1])
    bc = x.shape[0] // NCORES

    w = _pack_weights(np.asarray(W_ih1, np.float32), np.asarray(W_hh1, np.float32),
                      np.asarray(b_ih1, np.float32), np.asarray(b_hh1, np.float32),
                      np.asarray(W_ih2, np.float32), np.asarray(W_hh2, np.float32),
                      np.asarray(b_ih2, np.float32), np.asarray(b_hh2, np.float32),
                      np.asarray(fc1_w, np.float32), np.asarray(fc1_b, np.float32),
                      np.asarray(fc2_w, np.float32), np.asarray(fc2_b, np.float32))

    nc = _get_nc(t_steps, fut, bc, _loop_steps, int(_outer))
    in_maps = []
    for c in range(NCORES):
        m = dict(w)
        m["xe"] = _pack_x(x[c * bc : (c + 1) * bc, :t_steps], t_steps)
        in_maps.append(m)

    res = run_bass_kernel_spmd(nc, in_maps, core_ids=list(range(NCORES)), trace=_trace)
    outs = [res.results[c]["out"] for c in range(NCORES)]  # each [fut+1, bc]
    full = np.concatenate(outs, axis=1).T  # [B, fut+1]
    kernel._last_exec_ns = res.exec_time_ns
    return np.ascontiguousarray(full.astype(np.float32))
